# revision 26
# baseline (speedup 1.0000x reference)
"""Trainium2 Bass kernel for the Bahdanau-style attention scorer:

    scores[b, t] = v . tanh(X[b, t] @ WO^T + WG @ g[b])

Shapes: inputs [64, 4096, 128] f32, g [64, 128], WO/WG [256, 128], v [1, 256].
Output: [64, 4096] f32.

Strategy (data-parallel over batch, 8 NeuronCores), v4:
  - Host: cast X to bf16, pre-transpose to [B, D, T]; precompute term2
    C = g @ WG^T in f32.
  - Device, [s, t] orientation. The binding resource is the ACT engine:
    tanh throughput is 1 elem/lane/cycle at 1.2 GHz (65536 lane-cycles =
    54.6us/core) PLUS ~190ns fixed overhead per ACTIVATE instruction, so
    the chunking is chosen to minimize instruction count under the 8-bank
    PSUM budget: per (batch, half) the 4096 tokens are split (1024, 1536,
    1536) = 48 ACTIVATEs/core.
  - h-major term1 per batch keeps WO_h loaded across 8 matmuls; v-dot
    matmuls for batch b-1 are emitted between batch b's h0/h1 term1
    passes so ACT's psum backlog never drains; tanh output for a whole
    (batch, half) lands in one contiguous [128, 4096] bf16 SBUF tile so
    v-dot rhs slices never straddle tiles.
  - Scores pack 4 sub-chunks per PSUM bank via tile_position column
    tiling; the sync queue DMAs the 4 used rows straight from PSUM to
    DRAM in f32 (no DVE cast pass, 8KB per transfer).
  - Startup: the first X piece (scalar HWDGE) and WO^T (vector HWDGE)
    are hoisted into the NEFF preamble block on the two queues that
    reach the start barrier early, so both rings wake and the transfers
    complete during the ~7us preamble; everything else issues at block
    entry on the warmed rings.  A dummy tanh forces the ACT table load
    off the critical path; two short matmuls start the PE p-state ramp.
  - Tail: the last batch's v-dot stop-matmuls are emitted in
    column-availability order with per-bank PSUM->DRAM DMAs, and its h1
    chunk order is flipped so only ~2 small matmuls + one 8KB DMA
    remain after the final ACTIVATE.
  - Post-passes: preamble hoist, LdW dedup and semaphore cleanup.
"""

import numpy as np
import ml_dtypes

import concourse.bass as bass
import concourse.mybir as mybir
import concourse.tile as tile
from concourse import bacc
from concourse.bass_utils import run_bass_kernel_spmd

B, T, D, S = 64, 4096, 128, 256
N_CORES = 8
B_PER_CORE = B // N_CORES  # 8
MM_N = 512  # matmul moving free dim (one PSUM bank of f32)
CHUNKS = (1024, 1536, 1536)  # ACT chunk sizes per (batch, half)

_BF16 = ml_dtypes.bfloat16

_nc_cache = {}

# test.py reads this to get exec_time_ns from the traced run
LAST_RESULTS = None


def _tanh_table_id(nc):
    """Index (insertion order) of the first activation-function set that
    contains Tanh — the `act_func_set_id` for InstLoadActFuncSet."""
    from concourse.hw_specs import get_activation_tables

    for i, funcs in enumerate(get_activation_tables(nc.m.arch).values()):
        if mybir.ActivationFunctionType.Tanh in funcs:
            return i
    raise ValueError("no activation table containing Tanh")


def _build_bass():
    nc = bacc.Bacc("TRN2", target_bir_lowering=False)
    xt = nc.dram_tensor(
        "xt", [B_PER_CORE, D, T], mybir.dt.bfloat16, kind="ExternalInput"
    )
    wot = nc.dram_tensor("wot", [D, S], mybir.dt.bfloat16, kind="ExternalInput")
    # ct[p, h*B_PER_CORE + b] = C[b, h*128 + p]  (term2, f32)
    ct = nc.dram_tensor(
        "ct", [D, 2 * B_PER_CORE], mybir.dt.float32, kind="ExternalInput"
    )
    # vt[p, h] = v[h*128 + p]
    vt = nc.dram_tensor("vt", [D, 2], mybir.dt.bfloat16, kind="ExternalInput")
    # raw scores: rows {0,32,64,96} of each score bank, f32 straight from
    # PSUM; out[2b+half, k] = tokens [half*2048 + 512k : +512) of batch b
    out = nc.dram_tensor(
        "out", [2 * B_PER_CORE, 4, MM_N], mybir.dt.float32, kind="ExternalOutput"
    )

    with tile.TileContext(nc) as tc:
        with (
            tc.tile_pool(name="consts", bufs=1) as consts,
            tc.tile_pool(name="xin", bufs=5) as xin_pool,
            tc.tile_pool(name="tanh", bufs=6) as tanh_pool,
            tc.tile_pool(name="orow", bufs=3) as orow_pool,
            tc.tile_pool(name="ps1", bufs=2, space="PSUM") as ps1_pool,
            tc.tile_pool(name="ps2", bufs=2, space="PSUM") as ps2_pool,
        ):
            # Pre-place the Tanh table load as the first scalar-queue
            # instruction of the main block. Without this, the auto-pass
            # puts it in the preamble block (because the hoisted DMAs give
            # the scalar queue a preamble presence), where its 1.28us
            # engine time delays the scalar queue's start-barrier arrival
            # and thus block entry for every engine.
            nc.scalar.add_instruction(
                mybir.InstLoadActFuncSet(
                    name=nc.get_next_instruction_name(),
                    ins=[],
                    outs=[],
                    act_func_set_id=_tanh_table_id(nc),
                )
            )

            # The first X piece and wot are emitted first on the scalar
            # HWDGE queue; _hoist_preamble moves exactly these two DMAs
            # into the preamble block (that queue reaches the start
            # barrier ~1us before the barrier gate, so the issue cost is
            # mostly free and the ring wakes + transfers during the
            # preamble).
            x0 = xin_pool.tile([D, T], mybir.dt.bfloat16, tag="xb")
            nc.scalar.dma_start(x0[:, 0:512], xt[0, :, 0:512])
            wot_sb = consts.tile([D, S], mybir.dt.bfloat16)
            nc.scalar.dma_start(wot_sb[:], wot[:])
            ct_sb = consts.tile([D, 2 * B_PER_CORE], mybir.dt.float32)
            nc.scalar.dma_start(ct_sb[:], ct[:])
            # vt is tiny (512B): issued first on the sync queue it doubles
            # as the sync-ring warmer, so the b0 pieces below transfer at
            # full speed instead of crawling through the ring wake
            vt_sb = consts.tile([D, 2], mybir.dt.bfloat16)
            nc.sync.dma_start(vt_sb[:], vt[:])

            # scrap feeds the dummy tanh and the PE warm-up; kept small so
            # the DVE memset completes right after block entry
            scrap = consts.tile([128, 256], mybir.dt.bfloat16)
            nc.vector.memset(scrap[:], 0)

            # dummy tanh exercises the pre-placed table so the load is
            # charged at block entry, off the first real activation
            dummy = consts.tile([128, 8], mybir.dt.bfloat16)
            nc.scalar.activation(
                dummy[:], scrap[:, :8], mybir.ActivationFunctionType.Tanh,
                bias=0.0, scale=1.0,
            )

            # PE warm-up: two short dummy matmuls start the HAM clock
            # ramp while the first X piece is in flight
            warm_ps = ps2_pool.tile([128, MM_N], mybir.dt.float32, tag="sc")
            for _ in range(2):
                nc.tensor.matmul(
                    warm_ps[:, :256], scrap[:, :128], scrap[:, :256],
                    start=True, stop=True,
                )

            # th_ref[b][h] = [128, T] bf16 tanh tile; sc_ref[b] = (scA, scB)
            th_ref = {}
            sc_ref = {}

            def emit_term1_b0(x_b):
                # b0 is inflow-limited: interleave h0/h1 chunk-by-chunk
                # so every arrived X piece feeds TWO ACT chunks, riding
                # out the DMA-ring ramp without ACT gaps; fine first
                # chunks so the first ACT needs only the preamble piece
                th0 = tanh_pool.tile([128, T], mybir.dt.bfloat16, tag="th")
                th1 = tanh_pool.tile([128, T], mybir.dt.bfloat16, tag="th")
                col = 0
                for csz in (512, 512, 1536, 1536):
                    for h, th in ((0, th0), (1, th1)):
                        ps = ps1_pool.tile(
                            [128, max(CHUNKS)], mybir.dt.float32, tag="mm1"
                        )
                        for q in range(csz // MM_N):
                            nc.tensor.matmul(
                                ps[:, q * MM_N : (q + 1) * MM_N],
                                wot_sb[:, h * 128 : (h + 1) * 128],
                                x_b[:, col + q * MM_N : col + (q + 1) * MM_N],
                                start=True,
                                stop=True,
                            )
                        nc.scalar.activation(
                            th[:, col : col + csz],
                            ps[:, :csz],
                            mybir.ActivationFunctionType.Tanh,
                            bias=ct_sb[:, h * B_PER_CORE : h * B_PER_CORE + 1],
                            scale=1.0,
                        )
                    col += csz
                th_ref[0] = {0: th0, 1: th1}

            def emit_term1_half(b, x_b, h, chunks=CHUNKS):
                th = tanh_pool.tile([128, T], mybir.dt.bfloat16, tag="th")
                col = 0
                for csz in chunks:
                    ps = ps1_pool.tile(
                        [128, max(CHUNKS)], mybir.dt.float32, tag="mm1"
                    )
                    for q in range(csz // MM_N):
                        nc.tensor.matmul(
                            ps[:, q * MM_N : (q + 1) * MM_N],
                            wot_sb[:, h * 128 : (h + 1) * 128],
                            x_b[:, col + q * MM_N : col + (q + 1) * MM_N],
                            start=True,
                            stop=True,
                        )
                    nc.scalar.activation(
                        th[:, col : col + csz],
                        ps[:, :csz],
                        mybir.ActivationFunctionType.Tanh,
                        bias=ct_sb[:, h * B_PER_CORE + b : h * B_PER_CORE + b + 1],
                        scale=1.0,
                    )
                    col += csz
                th_ref.setdefault(b, {})[h] = th

            def emit_vdot_half(bb, h):
                # sub-chunk Q (512 tokens) -> bank A rows 32k (Q=k) or
                # bank B rows 32k (Q=4+k); accumulation group per row:
                # h0 starts, h1 stops
                if h == 0:
                    scA = ps2_pool.tile([128, MM_N], mybir.dt.float32, tag="sc")
                    scB = ps2_pool.tile([128, MM_N], mybir.dt.float32, tag="sc")
                    sc_ref[bb] = (scA, scB)
                scA, scB = sc_ref[bb]
                th = th_ref[bb][h]
                for k in range(4):
                    for sc, Q in ((scA, k), (scB, 4 + k)):
                        nc.tensor.matmul(
                            sc[32 * k : 32 * k + 1, :],
                            vt_sb[:, h : h + 1],
                            th[:, Q * MM_N : (Q + 1) * MM_N],
                            start=(h == 0),
                            stop=(h == 1),
                            tile_position=(0, 32 * k),
                        )

            def emit_out(bb):
                # DVE: rows {0,32,64,96} of each bank psum->sbuf (f32,
                # 4 lanes, same free-size cost as 128); sync HWDGE: 8KB
                # sbuf->dram
                scA, scB = sc_ref.pop(bb)
                th_ref.pop(bb)
                for half, sc in ((0, scA), (1, scB)):
                    so = orow_pool.tile([128, MM_N], mybir.dt.float32, tag="so")
                    nc.vector.tensor_copy(so[:], sc[:])
                    nc.sync.dma_start(out[2 * bb + half], so[0:128:32, :])

            for b in range(B_PER_CORE):
                x_b = x0 if b == 0 else xin_pool.tile(
                    [D, T], mybir.dt.bfloat16, tag="xb"
                )
                if b == 0:
                    # cols [0:512) hoisted into the preamble on the scalar
                    # ring together with wot; piece2 also rides the warm
                    # scalar ring (its issue precedes every TANH in queue
                    # order, so it cannot stall the tanh stream); the big
                    # late pieces go on the sync ring, boundaries aligned
                    # to the b0 chunk schedule
                    nc.scalar.dma_start(x_b[:, 512:1024], xt[b, :, 512:1024])
                    nc.sync.dma_start(x_b[:, 1024:2560], xt[b, :, 1024:2560])
                    nc.sync.dma_start(x_b[:, 2560:4096], xt[b, :, 2560:4096])
                else:
                    for jj in range(2):
                        nc.sync.dma_start(
                            x_b[:, jj * 2048 : (jj + 1) * 2048],
                            xt[b, :, jj * 2048 : (jj + 1) * 2048],
                        )

                if b == 0:
                    emit_term1_b0(x_b)
                else:
                    emit_term1_half(b, x_b, 0)
                    emit_vdot_half(b - 1, 0)
                    emit_term1_half(
                        b, x_b, 1,
                        chunks=(1536, 1536, 1024) if b == B_PER_CORE - 1
                        else CHUNKS,
                    )
                    emit_vdot_half(b - 1, 1)
                    emit_out(b - 1)

            # tail: h0 start-matmuls for the final batch, then h1 stop-
            # matmuls in column-availability order (its h1 chunks were
            # 1536,1536,1024: Q0-2 after chunk0, Q3-5 after chunk1, Q6-7
            # after chunk2), draining each PSUM bank the moment its last
            # sub-chunk stops so only ~2 matmuls + one DMA trail the
            # final ACTIVATE
            bb = B_PER_CORE - 1
            emit_vdot_half(bb, 0)
            scA, scB = sc_ref[bb]
            th1 = th_ref[bb][1]
            for k in range(4):
                nc.tensor.matmul(
                    scA[32 * k : 32 * k + 1, :],
                    vt_sb[:, 1:2],
                    th1[:, k * MM_N : (k + 1) * MM_N],
                    start=False,
                    stop=True,
                    tile_position=(0, 32 * k),
                )
            soA = orow_pool.tile([128, MM_N], mybir.dt.float32, tag="so")
            nc.vector.tensor_copy(soA[:], scA[:])
            nc.sync.dma_start(out[2 * bb], soA[0:128:32, :])
            for k in range(4):
                nc.tensor.matmul(
                    scB[32 * k : 32 * k + 1, :],
                    vt_sb[:, 1:2],
                    th1[:, (4 + k) * MM_N : (5 + k) * MM_N],
                    start=False,
                    stop=True,
                    tile_position=(0, 32 * k),
                )
            soB = orow_pool.tile([128, MM_N], mybir.dt.float32, tag="so")
            nc.vector.tensor_copy(soB[:], scB[:])
            nc.sync.dma_start(out[2 * bb + 1], soB[0:128:32, :])

    _hoist_preamble(nc)
    _dedup_ldweights(nc)
    _optimize_sems(nc)
    nc.compile()
    # compile()'s auto table-load pass inserts a redundant block-0 load
    # (function entry) even though our block-1 load dominates every
    # activation; the block-0 copy's 1.28us engine time delays the scalar
    # queue's start-barrier arrival, so drop it.
    b0 = nc.m.functions[0].blocks[0]
    b0.instructions[:] = [
        i for i in b0.instructions
        if not (isinstance(i, mybir.InstLoadActFuncSet)
                and not (i.sync_info and (i.sync_info.on_wait
                                          or i.sync_info.on_update)))
    ]
    return nc


def _hoist_preamble(nc):
    """Move the first wait-free DMA on the Activation and DVE queues (the
    first X piece and WO^T) from the tile-context block into the main
    block, so they issue during the ~7us NEFF startup preamble and their
    rings wake + transfer while the engines are still in the start
    barrier. Only those two queues reach the barrier with ~1us of slack,
    so hoisting there does not delay the barrier gate. The DMAs are
    wait-free; their semaphore updates fire once either way, just
    earlier, so all downstream waits remain correct."""
    blocks = nc.m.functions[0].blocks
    if len(blocks) < 2:
        return 0
    b0, b1 = blocks[0], blocks[1]
    ET = mybir.EngineType
    quota = {ET.Activation: 2}
    hoist = []
    for idx, inst in enumerate(b1.instructions):
        eng = getattr(inst, "engine", None)
        if eng not in quota or quota[eng] <= 0:
            continue
        si = getattr(inst, "sync_info", None)
        if si is not None and si.on_wait:
            continue
        if isinstance(inst, mybir.InstDMACopy):
            hoist.append(idx)
            quota[eng] -= 1
        if all(v <= 0 for v in quota.values()):
            break
    if not hoist:
        return 0
    moved = [b1.instructions[i] for i in hoist]
    keep = [x for i, x in enumerate(b1.instructions) if i not in set(hoist)]
    b1.instructions[:] = keep
    # insert after the leading InstCall so each engine queue sees the
    # hoisted work before its barrier drain
    pos = 1 if b0.instructions and isinstance(
        b0.instructions[0], mybir.InstCall
    ) else 0
    b0.instructions[pos:pos] = moved
    return len(moved)


def _dedup_ldweights(nc):
    """Drop an InstLdweights whose weights exactly match the still-loaded
    weights (no intervening PE weight change), so back-to-back same-weight
    matmuls can pipeline. A sync-carrying duplicate LdW is also dropped
    when its waits can move onto the immediately following matmul (same
    queue position, so semantics are identical)."""
    n_removed = 0
    for blk in nc.m.functions[0].blocks:
        insts = blk.instructions
        out = []
        last_key = None
        i = 0
        while i < len(insts):
            inst = insts[i]
            if isinstance(inst, mybir.InstLdweights):
                si = inst.sync_info
                has_wait = bool(si and si.on_wait)
                has_upd = bool(si and si.on_update)
                key = (
                    str(inst.ins[0]),
                    str(getattr(inst, "tile_position", None)),
                    str(getattr(inst, "perf_mode", None)),
                    str(getattr(inst, "is_transpose", None)),
                )
                if key == last_key and not has_upd:
                    if not has_wait:
                        n_removed += 1
                        i += 1
                        continue
                    nxt = insts[i + 1] if i + 1 < len(insts) else None
                    if (
                        isinstance(nxt, mybir.InstMatmult)
                        and getattr(nxt, "sync_info", None) is not None
                        and not nxt.sync_info.on_wait
                    ):
                        nxt.sync_info.on_wait[:] = list(si.on_wait)
                        n_removed += 1
                        i += 1
                        continue
                last_key = key
            elif isinstance(inst, mybir.InstMatmult):
                pass  # matmul does not clobber loaded weights
            elif getattr(inst, "engine", None) == mybir.EngineType.PE:
                if not isinstance(inst, mybir.InstEventSemaphore):
                    last_key = None
            out.append(inst)
            i += 1
        blk.instructions[:] = out
    return n_removed


def _optimize_sems(nc):
    """Reduce standalone EVENT_SEMAPHORE instructions on the engine
    queues: drop waits provably satisfied by same-queue program order,
    then fold standalone wait-only/update-only semaphore instructions
    into adjacent compute instructions with free sync slots."""
    n_dropped = n_merged = 0
    for blk in nc.m.functions[0].blocks:
        insts = blk.instructions
        owner = {}
        for inst in insts:
            si = getattr(inst, "sync_info", None)
            if si is None:
                continue
            eng = getattr(inst, "engine", None)
            tag = "DMA" if isinstance(inst, mybir.InstDMACopy) else eng
            for u in si.on_update or []:
                owner.setdefault(u.id, set()).add(tag)

        counts = {}
        for inst in insts:
            si = getattr(inst, "sync_info", None)
            eng = getattr(inst, "engine", None)
            if si is None or eng is None:
                continue
            cnt = counts.setdefault(eng, {})
            if si.on_wait:
                kept = []
                for w in si.on_wait:
                    if (
                        owner.get(w.id) == {eng}
                        and getattr(w, "wait_mode", None) == "sem-ge-imm"
                        and cnt.get(w.id, 0) >= (w.wait_value or 0)
                    ):
                        n_dropped += 1
                        continue
                    kept.append(w)
                if len(kept) != len(si.on_wait):
                    si.on_wait[:] = kept
            if not isinstance(inst, mybir.InstDMACopy):
                for u in si.on_update or []:
                    if u.update_mode == "sem-inc":
                        cnt[u.id] = cnt.get(u.id, 0) + 1
                    elif u.update_mode == "sem-add-imm":
                        cnt[u.id] = cnt.get(u.id, 0) + (u.update_value or 0)
                    else:
                        cnt[u.id] = -(10**9)

        mergeable = (
            mybir.InstActivation,
            mybir.InstMatmult,
            mybir.InstLdweights,
            mybir.InstTensorCopy,
            mybir.InstMemset,
        )
        next_on_engine = {}
        merged = set()
        for idx in range(len(insts) - 1, -1, -1):
            inst = insts[idx]
            eng = getattr(inst, "engine", None)
            si = getattr(inst, "sync_info", None)
            if eng is None:
                continue
            if isinstance(inst, mybir.InstEventSemaphore) and si is not None:
                nxt = next_on_engine.get(eng)
                if (
                    si.on_wait
                    and not si.on_update
                    and nxt is not None
                    and getattr(nxt, "sync_info", None) is not None
                    and not nxt.sync_info.on_wait
                    and isinstance(nxt, mergeable)
                ):
                    nxt.sync_info.on_wait[:] = list(si.on_wait)
                    merged.add(idx)
                    n_merged += 1
                    continue
                if not si.on_wait and not si.on_update:
                    merged.add(idx)
                    continue
            next_on_engine[eng] = inst
        if merged:
            insts[:] = [x for i, x in enumerate(insts) if i not in merged]

        prev_on_engine = {}
        dead = set()
        for idx, inst in enumerate(insts):
            eng = getattr(inst, "engine", None)
            si = getattr(inst, "sync_info", None)
            if eng is None:
                continue
            if isinstance(inst, mybir.InstEventSemaphore) and si is not None:
                prv = prev_on_engine.get(eng)
                if (
                    si.on_update
                    and not si.on_wait
                    and prv is not None
                    and getattr(prv, "sync_info", None) is not None
                    and not prv.sync_info.on_update
                    and isinstance(prv, mergeable)
                ):
                    prv.sync_info.on_update[:] = list(si.on_update)
                    dead.add(idx)
                    n_merged += 1
                    continue
            prev_on_engine[eng] = inst
        if dead:
            insts[:] = [x for i, x in enumerate(insts) if i not in dead]
    return n_dropped, n_merged


def kernel(inputs, g, WO, WG, v):
    global LAST_RESULTS
    inputs = np.asarray(inputs, dtype=np.float32)
    g = np.asarray(g, dtype=np.float32)
    WO = np.asarray(WO, dtype=np.float32)
    WG = np.asarray(WG, dtype=np.float32)
    v = np.asarray(v, dtype=np.float32)

    # term2 (tiny): C[b, s] = g[b] @ WG[s]^T
    C_all = g @ WG.T  # [B, S] f32

    # X^T per batch: [B, D, T], bf16, contiguous
    x_bf = inputs.astype(_BF16)
    xt_all = np.ascontiguousarray(x_bf.transpose(0, 2, 1))  # [B, D, T]

    wot_host = np.ascontiguousarray(WO.T).astype(_BF16)  # [D, S]
    vt_host = np.ascontiguousarray(v.reshape(2, 128).T).astype(_BF16)  # [128, 2]

    in_maps = []
    for c in range(N_CORES):
        Cc = C_all[c * B_PER_CORE : (c + 1) * B_PER_CORE]  # [8, 256]
        ct_host = np.ascontiguousarray(
            Cc.reshape(B_PER_CORE, 2, 128).transpose(2, 1, 0).reshape(128, 2 * B_PER_CORE)
        ).astype(np.float32)
        in_maps.append(
            {
                "xt": xt_all[c * B_PER_CORE : (c + 1) * B_PER_CORE],
                "wot": wot_host,
                "ct": ct_host,
                "vt": vt_host,
            }
        )

    if "nc" not in _nc_cache:
        _nc_cache["nc"] = _build_bass()
    nc = _nc_cache["nc"]

    res = run_bass_kernel_spmd(nc, in_maps, list(range(N_CORES)))
    LAST_RESULTS = res
    outs = []
    for r in res.results:
        raw = np.asarray(r["out"], dtype=np.float32)  # [16, 4, 512]
        # row-pair (2b, 2b+1): bank A = tokens 0..2047, bank B = 2048..4095
        outs.append(raw.reshape(B_PER_CORE, T))
    return np.concatenate(outs, axis=0)
